# revision 33
# baseline (speedup 1.0000x reference)
"""LSQ quantizer forward kernel for Trainium2 (8 NeuronCores, data-parallel).

Computes out = (round(clip((x @ H) / s, -Qn, Qp)) * s) @ H.T for
x [4, 4096, 2048] f32, H [2048, 2048] f32, sharding the 16384 token rows
across 8 cores (2048 rows each).

Math:
- H = Syl * diag(signs) / sqrt(D) with Syl the Sylvester-Hadamard matrix.
  The 1/sqrt(D) and signs fold into the quant scale (applied to x on the
  host) and into sign-folded constants:
    v = (x/(s*sqrt(D))) @ (Syl . diag(signs))     (phase 1, fp16)
    q = clip(round(v), -8, 7)
    out = (s/sqrt(D)) * q @ (diag(signs) . Syl)   (phase 2, fp8)
  The final (s/sqrt(D)) is applied on the host after gathering (the
  device emits integer-valued fp16).
- Phase 1 Kronecker: Syl_2048 = H8 (x) Syl_256: three radix-2 butterfly
  stages (DVE, fp16, exact) + block-diagonal 256-contraction fp16 matmul.
- Quant: ACT u = Relu(v + 8) -> int8 (the int8 convert is the
  round-to-nearest-even); GPSIMD q = min(u,15) - 8 -> fp8e4.
- Phase 2 column split (psum-bank exact):
  * out cols {0:512} u {1024:1536}: k2=1 pairs via Syl_2048 = H2 (x)
    Syl_1024: r_g = q_g @ SylS_1024[:, :512] (fp8 DR), ACT evac to fp16,
    one H2 butterfly stage on DVE: [r0+r1, r0-r1].
  * out cols {512:1024} u {1536:2048}: k2=0 direct full-2048-K fp8 DR
    matmul, plain evac (ACT/Pool), no butterfly.
- Scheduling: strips of [256,512,512,512,256] rows; PE runs strip st's
  phase-2 interleaved with strip st+1's phase-1 so quant latency never
  stalls it. Output DMAs ride the GPSIMD DGE queue.
"""
import numpy as np
import ml_dtypes
from contextlib import ExitStack

import concourse.bacc as bacc
import concourse.mybir as mybir
import concourse.tile as tile
from concourse.bass_utils import run_bass_kernel_spmd

F16 = mybir.dt.float16
F32 = mybir.dt.float32
F8 = mybir.dt.float8e4
I8 = mybir.dt.int8

N_CORES = 8
D = 2048
ROWS_TOTAL = 4 * 4096
M_ROWS = ROWS_TOTAL // N_CORES   # 2048 rows per core

WIDTHS = [256] * 8
assert sum(WIDTHS) == M_ROWS
N_STRIPS = len(WIDTHS)
G = 8                            # phase-1 butterfly groups (H8)
U = D // G                       # 256 contraction per group
UT = U // 128                    # 2 contraction tiles per group
SYL8_COLS = 1536                 # 512 (k2=1 half) + 1024 (k2=0 direct)


def _build_kernel():
    nc = bacc.Bacc(trn_type="TRN2")

    xt_d = nc.dram_tensor("xt", [128, 16, M_ROWS], F16, kind="ExternalInput")
    sylp1_d = nc.dram_tensor("sylp1", [128, UT, D], F16, kind="ExternalInput")
    syl8_d = nc.dram_tensor("syl8", [128, 16, SYL8_COLS], F8,
                            kind="ExternalInput")
    out_d = nc.dram_tensor("out", [M_ROWS, D], F16, kind="ExternalOutput")

    AOT = mybir.AluOpType

    with tile.TileContext(nc) as tc:
        with ExitStack() as ctx:
            cpool = ctx.enter_context(tc.tile_pool(name="consts", bufs=1))
            xpool = ctx.enter_context(tc.tile_pool(name="xs", bufs=2))

            m_off = [sum(WIDTHS[:s]) for s in range(N_STRIPS + 1)]
            xs_tiles = [
                xpool.tile([128, 16, WIDTHS[s]], F16, tag=f"xs{s % 2}",
                           name=f"xs{s}")
                for s in range(N_STRIPS)
            ]
            nc.sync.dma_start(xs_tiles[0][:, 0:8], xt_d[:, 0:8, 0:m_off[1]])
            nc.sync.dma_start(xs_tiles[0][:, 8:16], xt_d[:, 8:16, 0:m_off[1]])
            sylp1_sb = cpool.tile([128, UT, D], F16, tag="sylp1")
            # only the first groups' phase-1 stationary races the x strips;
            # everything else queues behind so xs(0) gets the bandwidth
            nc.sync.dma_start(sylp1_sb[:, :, 0:512], sylp1_d[:, :, 0:512])
            nc.sync.dma_start(xs_tiles[1][:, 0:8], xt_d[:, 0:8, m_off[1]:m_off[2]])
            nc.sync.dma_start(xs_tiles[1][:, 8:16], xt_d[:, 8:16, m_off[1]:m_off[2]])
            nc.sync.dma_start(sylp1_sb[:, :, 512:2048], sylp1_d[:, :, 512:2048])
            syl8_sb = cpool.tile([128, 16, SYL8_COLS], F8, tag="syl8")
            # k2=1 half first (first phase-2 blocks need it), then k2=0
            nc.sync.dma_start(syl8_sb[:, :, 0:512], syl8_d[:, :, 0:512])
            nc.sync.dma_start(syl8_sb[:, :, 512:1024], syl8_d[:, :, 512:1024])
            nc.sync.dma_start(syl8_sb[:, :, 1024:1536],
                              syl8_d[:, :, 1024:1536])
            bias_t = cpool.tile([128, 1], F32, tag="bias")
            nc.vector.memset(bias_t[:], 8.0)

            v1pool = ctx.enter_context(tc.tile_pool(name="v1", bufs=1))
            v2pool = ctx.enter_context(tc.tile_pool(name="v2", bufs=1))
            v3pool = ctx.enter_context(tc.tile_pool(name="v3", bufs=2))
            qipool = ctx.enter_context(tc.tile_pool(name="q8i", bufs=2))
            qpool = ctx.enter_context(tc.tile_pool(name="q8", bufs=2))
            rrpool = ctx.enter_context(tc.tile_pool(name="rr", bufs=3))
            odpool = ctx.enter_context(tc.tile_pool(name="od", bufs=3))
            opool = ctx.enter_context(tc.tile_pool(name="o", bufs=3))
            ps1_pool = ctx.enter_context(
                tc.tile_pool(name="ps1", bufs=4, space="PSUM")
            )
            p2_pool = ctx.enter_context(
                tc.tile_pool(name="p2", bufs=1, space="PSUM")
            )

            def bfly(st):
                xs = xs_tiles[st]
                w = WIDTHS[st]
                v1 = v1pool.tile([128, 16, w], F16, tag="v1", name=f"v1_{st}")
                v2 = v2pool.tile([128, 16, w], F16, tag="v2", name=f"v2_{st}")
                v3 = v3pool.tile([128, 16, w], F16, tag="v3", name=f"v3_{st}")

                def st2(h, op):
                    o8, o2 = h * 8, (0 if op == "a" else 4)
                    f = (nc.vector.tensor_add if op == "a"
                         else nc.vector.tensor_sub)
                    f(v2[:, o8 + o2:o8 + o2 + 4, :],
                      v1[:, o8:o8 + 4, :], v1[:, o8 + 4:o8 + 8, :])

                def st3(q4, op):
                    o4, o2 = q4 * 4, (0 if op == "a" else 2)
                    f = (nc.vector.tensor_add if op == "a"
                         else nc.vector.tensor_sub)
                    f(v3[:, o4 + o2:o4 + o2 + 2, :],
                      v2[:, o4:o4 + 2, :], v2[:, o4 + 2:o4 + 4, :])

                nc.vector.tensor_add(v1[:, 0:8, :], xs[:, 0:8, :],
                                     xs[:, 8:16, :])
                st2(0, "a"); st3(0, "a"); st3(0, "s")
                st2(0, "s"); st3(1, "a"); st3(1, "s")
                nc.vector.tensor_sub(v1[:, 8:16, :], xs[:, 0:8, :],
                                     xs[:, 8:16, :])
                st2(1, "a"); st3(2, "a"); st3(2, "s")
                st2(1, "s"); st3(3, "a"); st3(3, "s")
                return v3

            def p1_slab(st, slab, v3, q8i, q8):
                w = WIDTHS[st]
                ps1 = ps1_pool.tile([128, 2, w], F32, tag="ps1")
                for half in range(2):
                    cb = slab * 2 + half
                    gg, nn = cb // 2, cb % 2
                    for kk in range(UT):
                        nc.tensor.matmul(
                            ps1[:, half, :],
                            sylp1_sb[:, kk, gg * U + nn * 128:
                                     gg * U + nn * 128 + 128],
                            v3[:, gg * UT + kk, :],
                            start=(kk == 0), stop=(kk == UT - 1),
                        )
                qisl = q8i[:, slab * 2:slab * 2 + 2, :]
                qsl = q8[:, slab * 2:slab * 2 + 2, :]
                nc.scalar.activation(
                    qisl, ps1[:], mybir.ActivationFunctionType.Relu,
                    bias=bias_t[:], scale=1.0,
                )
                nc.gpsimd.tensor_scalar(
                    out=qsl, in0=qisl, scalar1=15.0, scalar2=8.0,
                    op0=AOT.min, op1=AOT.subtract,
                )

            def p2_ms(st, ms, q8, d_on_pool, d_first=False):
                row0 = m_off[st] + ms * 128

                def r_part():
                    # k2=1 pair: r_g = q_g @ SylS1024[:, :512]
                    r01 = p2_pool.tile([128, 2, 512], F32, tag="r01")
                    for g in range(2):
                        for kk in range(4):
                            t = g * 8 + 2 * kk
                            nc.tensor.matmul(
                                r01[:, g, :],
                                q8[:, t:t + 2, ms * 128:(ms + 1) * 128],
                                syl8_sb[:, t:t + 2, 0:512],
                                start=(kk == 0), stop=(kk == 3),
                                perf_mode=mybir.MatmulPerfMode.DoubleRow,
                            )
                    rr = rrpool.tile([128, 2, 512], F16, tag="rr")
                    nc.scalar.copy(rr[:], r01[:])
                    # H2 butterfly on DVE
                    o = opool.tile([128, 2, 512], F16, tag="o")
                    nc.vector.tensor_add(o[:, 0, :], rr[:, 0, :], rr[:, 1, :])
                    nc.vector.tensor_sub(o[:, 1, :], rr[:, 0, :], rr[:, 1, :])
                    nc.sync.dma_start(out_d[row0:row0 + 128, 0:512],
                                        o[:, 0, :])
                    nc.sync.dma_start(out_d[row0:row0 + 128, 1024:1536],
                                        o[:, 1, :])

                def d_part():
                    # k2=0 direct: out cols {512:1024, 1536:2048}
                    dps = p2_pool.tile([128, 2, 512], F32, tag="dps")
                    for j in range(2):
                        for kk in range(8):
                            t = 2 * kk
                            nc.tensor.matmul(
                                dps[:, j, :],
                                q8[:, t:t + 2, ms * 128:(ms + 1) * 128],
                                syl8_sb[:, t:t + 2,
                                        512 + j * 512:1024 + j * 512],
                                start=(kk == 0), stop=(kk == 7),
                                perf_mode=mybir.MatmulPerfMode.DoubleRow,
                            )
                    od = odpool.tile([128, 2, 512], F16, tag="od")
                    nc.scalar.copy(od[:], dps[:])
                    nc.sync.dma_start(out_d[row0:row0 + 128, 512:1024],
                                        od[:, 0, :])
                    nc.sync.dma_start(out_d[row0:row0 + 128, 1536:2048],
                                        od[:, 1, :])

                if d_first:
                    d_part(); r_part()
                else:
                    r_part(); d_part()

            # ---- software-pipelined main loop: PE one strip ahead ----
            qtiles = {}

            def p1_strip(st):
                v3 = bfly(st)
                w = WIDTHS[st]
                q8i = qipool.tile([128, 16, w], I8, tag="q8i",
                                  name=f"q8i_{st}")
                q8 = qpool.tile([128, 16, w], F8, tag="q8", name=f"q8_{st}")
                qtiles[st] = q8
                return v3, q8i, q8

            v3, q8i, q8 = p1_strip(0)
            for slab in range(8):
                p1_slab(0, slab, v3, q8i, q8)

            for st in range(1, N_STRIPS):
                if st + 1 < N_STRIPS:
                    nc.sync.dma_start(
                        xs_tiles[st + 1][:, 0:8],
                        xt_d[:, 0:8, m_off[st + 1]:m_off[st + 2]],
                    )
                    nc.sync.dma_start(
                        xs_tiles[st + 1][:, 8:16],
                        xt_d[:, 8:16, m_off[st + 1]:m_off[st + 2]],
                    )
                v3, q8i, q8 = p1_strip(st)
                prev_q8 = qtiles[st - 1]
                nms_prev = WIDTHS[st - 1] // 128
                # interleave prev strip's phase-2 with this strip's phase-1
                slabs_per_ms = 8 // nms_prev
                slab = 0
                for ms in range(nms_prev):
                    for _ in range(slabs_per_ms // 2):
                        p1_slab(st, slab, v3, q8i, q8)
                        slab += 1
                    p2_ms(st - 1, ms, prev_q8, d_on_pool=False)
                    for _ in range(slabs_per_ms - slabs_per_ms // 2):
                        p1_slab(st, slab, v3, q8i, q8)
                        slab += 1
                while slab < 8:
                    p1_slab(st, slab, v3, q8i, q8)
                    slab += 1

            st = N_STRIPS - 1
            for ms in range(WIDTHS[st] // 128):
                p2_ms(st, ms, qtiles[st], d_on_pool=False)

    nc.finalize()
    return nc


def _make_syl(n):
    h = np.array([[1.0]], dtype=np.float32)
    while h.shape[0] < n:
        h = np.block([[h, h], [h, -h]])
    return h


_CACHE = {}
_CONSTS = {}


def _host_consts(hadamard):
    h = np.asarray(hadamard, dtype=np.float32)
    sqd = np.float32(np.sqrt(np.float32(D)))
    signs = np.sign(h[0, :] * sqd).astype(np.float32)
    key = signs.tobytes()
    if key in _CONSTS:
        return _CONSTS[key]
    syl = h * sqd * signs[None, :]          # pure Sylvester, +-1
    syl256 = _make_syl(U)
    sylp1 = np.concatenate(
        [syl256 * signs[g * U:(g + 1) * U][None, :] for g in range(G)],
        axis=1,
    ).astype(np.float16)                     # [U, D]
    sylp1 = np.ascontiguousarray(
        sylp1.reshape(UT, 128, D).transpose(1, 0, 2)
    )
    # phase-2 moving: row-signed Syl columns [0:1024] + [1536:2048]
    ssyl = signs[:, None] * syl
    syl8 = np.concatenate([ssyl[:, 0:1024], ssyl[:, 1536:2048]], axis=1)
    syl8 = syl8.astype(ml_dtypes.float8_e4m3)
    syl8 = np.ascontiguousarray(
        syl8.reshape(16, 128, SYL8_COLS).transpose(1, 0, 2)
    )
    _CONSTS[key] = (sylp1, syl8)
    return _CONSTS[key]


def _prep_in_maps(x, hadamard, inv_sp):
    sylp1, syl8 = _host_consts(hadamard)
    xf = (np.asarray(x, dtype=np.float32).reshape(ROWS_TOTAL, D)
          * np.float32(inv_sp))
    in_maps = []
    for c in range(N_CORES):
        xs = np.ascontiguousarray(
            xf[c * M_ROWS:(c + 1) * M_ROWS].T.astype(np.float16)
            .reshape(16, 128, M_ROWS).transpose(1, 0, 2)
        )
        in_maps.append({"xt": xs, "sylp1": sylp1, "syl8": syl8})
    return in_maps


def kernel(x, scale, hadamard, Qn, Qp, num_elements):
    scale_f = np.float32(np.asarray(scale).reshape(-1)[0])
    qn = float(np.asarray(Qn))
    qp = float(np.asarray(Qp))
    ne = float(np.asarray(num_elements))
    assert qn == 8.0 and qp == 7.0

    gs = np.float32(1.0) / np.sqrt(np.float32(ne) * np.float32(qp))
    bw = scale_f * gs
    s = (scale_f - bw) + bw
    sqd = np.float32(np.sqrt(np.float32(D)))
    inv_sp = float(np.float32(1.0) / (s * sqd))
    out_scale = np.float32(s / sqd)

    if "nc" not in _CACHE:
        _CACHE["nc"] = _build_kernel()
    nc = _CACHE["nc"]

    in_maps = _prep_in_maps(x, hadamard, inv_sp)
    res = run_bass_kernel_spmd(nc, in_maps, core_ids=list(range(N_CORES)))
    out = np.concatenate(
        [np.asarray(res.results[c]["out"]).astype(np.float32)
         for c in range(N_CORES)], axis=0
    ) * out_scale
    return out.reshape(np.asarray(x).shape)


def profile_once(inputs):
    """Return HW exec time in ns via NTFF if available, else TimelineSim."""
    nc = _CACHE["nc"]
    try:
        ne = float(np.asarray(inputs["num_elements"]))
        scale_f = np.float32(np.asarray(inputs["scale"]).reshape(-1)[0])
        gs = np.float32(1.0) / np.sqrt(np.float32(ne) * np.float32(7.0))
        bw = scale_f * gs
        s = (scale_f - bw) + bw
        sqd = np.float32(np.sqrt(np.float32(D)))
        in_maps = _prep_in_maps(
            inputs["x"], inputs["hadamard"],
            float(np.float32(1.0) / (s * sqd)),
        )
        res = run_bass_kernel_spmd(
            nc, in_maps, core_ids=list(range(N_CORES)), trace=True,
        )
        if res.exec_time_ns is not None:
            return res.exec_time_ns
    except Exception:
        pass
    from concourse.timeline_sim import TimelineSim

    return int(TimelineSim(nc, trace=False).simulate())
